# revision 49
# baseline (speedup 1.0000x reference)
"""Trainium2 Bass/Tile kernel for nn_CrossAttentionFiLM — factored attention.

Math (see reference):
    gamma = z @ Wg.T + bg ; beta = z @ Wb.T + bb
    out1  = LN1(x @ Wl.T + bl) * (1+gamma) + beta
    q     = (out1 @ Wq.T + bq) / sqrt(d)            [B, H, d]
    scores= qk . tf with qk = Wk_h^T q  (k never formed)
    attn  = softmax(scores)
    amix  = sum_t attn * tf ; ctx = Wv_h @ amix ; (+bv via folded bias)
    out   = gelu(out1 + LN2(ctx @ Wo.T + bo'))      bo' = bo + Wo@bv

Implementation notes (v2):
  - text_feat streams twice: tfc [c,t] bf16 (scores) on the sync HWDGE
    ring, tft [t,c] fp8 (amix) on the gpsimd SWDGE ring; weights and
    outputs ride the scalar HWDGE ring. Decoupled FIFOs keep the bulk
    stream saturated while the front chain's weights load.
  - biases are accumulated in PSUM by appending a rank-1 ones-matmul to
    each accumulation chain (no [128,F] broadcast DMAs).
  - LN gain/bias vectors are DMA-broadcast once and kept resident.
  - tail computes ctxT = Wv_h^T-contracted amix directly in [f, b]
    orientation (no PE transposes), runs Wo with ctxT stationary, and is
    split into two 64-sample halves so the first overlaps the group loop.
  - attention-weight transposes run in bf16 (quarter-bank PSUM tiles).

Sharding: pure data parallel over batch, B=1024 -> 128 rows per core x 8.
"""

import os
import sys

for _p in ("/opt/trn_rl_repo",):
    if os.path.isdir(_p) and _p not in sys.path:
        sys.path.append(_p)

import numpy as np
import ml_dtypes

os.environ.setdefault("JAX_COMPILATION_CACHE_DIR", "/tmp/jax_comp_cache")

import concourse.bass as bass
import concourse.tile as tile
from concourse import bacc, mybir
from concourse.bass_utils import run_bass_kernel_spmd
from concourse.masks import make_identity

BF16 = mybir.dt.bfloat16
FP8 = mybir.dt.float8e4
F32 = mybir.dt.float32
I32 = mybir.dt.int32
AF = mybir.ActivationFunctionType
ALU = mybir.AluOpType

B, T, F_DIM, Z_DIM, TXT_DIM, H = 1024, 128, 1024, 256, 768, 8
D = F_DIM // H  # 128
NCORES = 8
BC = B // NCORES  # 128 batch rows per core
EPS = 1e-5
CC_Z = Z_DIM // 128  # 2
CC_F = F_DIM // 128  # 8
CC_C = TXT_DIM // 128  # 6
NG = 8  # sample groups per core
GS = BC // NG  # 16 samples per group
HB = BC // 2  # tail half size (64 samples)

M0 = 16.0  # fixed softmax exp shift; |scores| ~ 12 max for this data

# bias table rows (all [F_DIM] f32 vectors; rank-1 matmul accumulated)
(IB_BG1, IB_BB, IB_BL, IB_BQS, IB_BO) = range(5)
# gbt rows (bf16, broadcast resident): ln1_g, ln1_b, ln2_g, ln2_b
(IG_G1, IG_B1, IG_G2, IG_B2) = range(4)


def build(nc, with_mask=False):
    """Declare per-core DRAM I/O and emit the Tile program."""
    xt = nc.dram_tensor("xt", [128, CC_F, BC], BF16, kind="ExternalInput").ap()
    zt = nc.dram_tensor("zt", [128, CC_Z, BC], BF16, kind="ExternalInput").ap()
    tfc = nc.dram_tensor("tfc", [NG, 2, 128, CC_C, GS // 2, T], BF16,
                         kind="ExternalInput").ap()
    tft = nc.dram_tensor("tft", [NG, 2, 128, GS // 2, TXT_DIM], FP8,
                         kind="ExternalInput").ap()
    wg = nc.dram_tensor("wg", [128, CC_Z, F_DIM], BF16, kind="ExternalInput").ap()
    wb = nc.dram_tensor("wb", [128, CC_Z, F_DIM], BF16, kind="ExternalInput").ap()
    wl = nc.dram_tensor("wl", [128, CC_F, F_DIM], BF16, kind="ExternalInput").ap()
    wq = nc.dram_tensor("wq", [128, CC_F, F_DIM], BF16, kind="ExternalInput").ap()
    wo = nc.dram_tensor("wo", [128, CC_F, F_DIM], BF16, kind="ExternalInput").ap()
    wkh = nc.dram_tensor("wkh", [128, H, TXT_DIM], BF16,
                         kind="ExternalInput").ap()
    wvt = nc.dram_tensor("wvt", [128, CC_C, F_DIM], BF16,
                         kind="ExternalInput").ap()
    bias = nc.dram_tensor("bias", [5, F_DIM], BF16, kind="ExternalInput").ap()
    gbt = nc.dram_tensor("gbt", [4, F_DIM], BF16, kind="ExternalInput").ap()
    mbg = None
    if with_mask:
        mbg = nc.dram_tensor("mbg", [NG, 128, 4 * T], F32,
                             kind="ExternalInput").ap()
    out = nc.dram_tensor("out", [BC, F_DIM], F32, kind="ExternalOutput").ap()

    with tile.TileContext(nc) as tc:
        _emit(nc, tc, xt, zt, tfc, tft, att_mbg=mbg, wg=wg, wb=wb, wl=wl,
              wq=wq, wo=wo, wkh=wkh, wvt=wvt, bias=bias, gbt=gbt, out=out)
    return nc


def _emit(nc, tc, xt, zt, tfc, tft, att_mbg, wg, wb, wl, wq, wo, wkh, wvt,
          bias, gbt, out):
    from contextlib import ExitStack

    ctxmgr = ExitStack()
    with ctxmgr:
        singles = ctxmgr.enter_context(tc.tile_pool(name="singles", bufs=1))
        scratch = ctxmgr.enter_context(tc.tile_pool(name="scratch", bufs=3))
        s16p = ctxmgr.enter_context(tc.tile_pool(name="s16p", bufs=4))
        wstream = ctxmgr.enter_context(tc.tile_pool(name="wstream", bufs=2))
        tfcp = ctxmgr.enter_context(tc.tile_pool(name="tfcp", bufs=2))
        tftp = ctxmgr.enter_context(tc.tile_pool(name="tftp", bufs=2))
        attp = ctxmgr.enter_context(tc.tile_pool(name="attp", bufs=2))
        # PSUM: 8 banks: "big" [128,1024]x2bufs = 4 banks, four single-bank
        # score tags "s0".."s3" (concurrent accumulation chains must live in
        # different banks).  bf16 PE transposes share the "s*" slots.
        ps = ctxmgr.enter_context(tc.tile_pool(name="ps", bufs=2, space="PSUM"))

        # ---- DMA ordering: everything front-critical rides the scalar
        # HWDGE queue as ONE priority FIFO (SDMA round-robin is packet-fair,
        # so a concurrent bulk stream would starve the small weight loads).
        # Groups 0/1 also load from the scalar queue; groups 2+ ride the
        # sync (tfc) and gpsimd (tft) rings, naturally gated by the tile
        # pools' bufs=2 recycling so they cannot start before group-0 is
        # consumed. ----
        # text_feat streams in HALF-group (8-sample) slices: finer pool
        # gating keeps the bulk queues saturated and PE idle gaps short.
        HGS = GS // 2

        def emit_dma(k, eng_c=None, eng_t=None):
            g, hf = k // 2, k % 2
            tfc_t = tfcp.tile([128, CC_C, HGS, T], BF16, tag="tfc", bufs=3,
                              name="tfc_t")
            (eng_c or nc.sync).dma_start(out=tfc_t, in_=tfc[g][hf])
            tft_t = tftp.tile([128, HGS, TXT_DIM], FP8, tag="tft", bufs=3,
                              name="tft_t")
            (eng_t or nc.gpsimd).dma_start(out=tft_t, in_=tft[g][hf])
            return tfc_t, tft_t

        # bias rows live at 32-aligned partitions (PE operand alignment):
        # row i -> partition 32*(i%4), free slot i//4
        bias_sb = singles.tile([128, 2, F_DIM], BF16)
        nc.scalar.dma_start(
            out=bias_sb[0:97:32, 0, :],
            in_=bias[0:4, :])
        nc.scalar.dma_start(out=bias_sb[0:1, 1, :], in_=bias[4:5, :])
        zt_sb = singles.tile([128, CC_Z, BC], BF16)
        nc.scalar.dma_start(out=zt_sb, in_=zt)
        wg_sb = singles.tile([128, CC_Z, F_DIM], BF16)
        nc.scalar.dma_start(out=wg_sb, in_=wg)
        wb_sb = singles.tile([128, CC_Z, F_DIM], BF16)
        nc.scalar.dma_start(out=wb_sb, in_=wb)
        xt_sb = singles.tile([128, CC_F, BC], BF16)
        nc.scalar.dma_start(out=xt_sb, in_=xt)
        wkh_sb = singles.tile([128, H, TXT_DIM], BF16)
        nc.scalar.dma_start(out=wkh_sb, in_=wkh)
        # ln2 g/b stay resident; ln1 g/b ride transient s16p tiles (SBUF)
        gbt_sb = singles.tile([128, 2, F_DIM], BF16)
        g2 = gbt[2:4, :]
        nc.scalar.dma_start(
            out=gbt_sb,
            in_=bass.AP(tensor=g2.tensor, offset=g2.offset,
                        ap=[[0, 128]] + list(g2.ap)))
        g1b1_t = s16p.tile([128, 2, F_DIM], BF16, tag="g1b1", bufs=1,
                           name="g1b1_t")
        g1 = gbt[0:2, :]
        nc.scalar.dma_start(
            out=g1b1_t,
            in_=bass.AP(tensor=g1.tensor, offset=g1.offset,
                        ap=[[0, 128]] + list(g1.ap)))
        wl_sb = singles.tile([128, CC_F, F_DIM], BF16)
        nc.scalar.dma_start(out=wl_sb, in_=wl)
        wq_sb = singles.tile([128, CC_F, F_DIM], BF16)
        nc.scalar.dma_start(out=wq_sb, in_=wq)
        ident = singles.tile([128, 128], F32)
        make_identity(nc, ident)
        identb = singles.tile([128, 128], BF16)
        nc.gpsimd.tensor_copy(identb, ident)
        eps_t = singles.tile([128, 1], F32)
        nc.vector.memset(eps_t, EPS)
        negm0_t = singles.tile([128, 1], F32)
        nc.vector.memset(negm0_t, -M0)
        ones_t = singles.tile([128, 128], BF16)
        nc.vector.memset(ones_t, 1.0)

        def bias_mm(ps_t, row, part=slice(0, 128)):
            """Accumulate bias row into an open PSUM chain (rank-1 matmul)."""
            n = part.stop - part.start
            p0 = 32 * (row % 4)
            for nh in range(2):
                nc.tensor.matmul(
                    ps_t[part, nh * 512:(nh + 1) * 512],
                    lhsT=ones_t[p0:p0 + 1, 0:n],
                    rhs=bias_sb[p0:p0 + 1, row // 4,
                                nh * 512:(nh + 1) * 512],
                    start=False, stop=True,
                    tile_position=(p0, part.start))

        # persistent activations
        out1 = singles.tile([BC, F_DIM], F32)
        out1t = singles.tile([128, CC_F, BC], BF16)
        qt_sb = singles.tile([128, H, BC], BF16)
        qkT_sb = singles.tile([128, CC_C, H, BC], BF16)
        # amixT is kept b-major (cc, b, h) so the per-group PSUM->SBUF copy
        # is contiguous; the tail's matmul reads it with an 8-stride AP.
        amixT_sb = singles.tile([128, CC_C, BC, H], BF16)

        # ---- FiLM params: gamma1 = z@Wg.T + (1+bg), beta = z@Wb.T + bb ----
        film16 = {}
        for w_sb, brow, nm in ((wg_sb, IB_BG1, "gamma1"), (wb_sb, IB_BB, "beta")):
            ps_t = ps.tile([BC, F_DIM], F32, tag="big", name="film_ps")
            for cc in range(CC_Z):
                for nh in range(2):
                    nc.tensor.matmul(
                        ps_t[:, nh * 512:(nh + 1) * 512],
                        lhsT=zt_sb[:, cc, :],
                        rhs=w_sb[:, cc, nh * 512:(nh + 1) * 512],
                        start=(cc == 0), stop=False)
            bias_mm(ps_t, brow)
            f16 = s16p.tile([BC, F_DIM], BF16, tag="a16", name=nm)
            nc.vector.tensor_copy(f16, ps_t)
            film16[nm] = f16

        # FiLM-folded LN1 gain/bias: G = g1*(1+gamma), Bv = b1*(1+gamma)+beta
        G_t = s16p.tile([BC, F_DIM], BF16, tag="a16", name="G_t")
        nc.vector.tensor_mul(G_t, film16["gamma1"], g1b1_t[:, 0, :])
        B_t = s16p.tile([BC, F_DIM], BF16, tag="a16", name="B_t")
        nc.vector.tensor_mul(B_t, film16["gamma1"], g1b1_t[:, 1, :])
        nc.vector.tensor_add(B_t, B_t, film16["beta"])

        # ---- h1 = x@Wl.T + bl (PSUM-resident) ----
        h1_ps = ps.tile([BC, F_DIM], F32, tag="big", name="h1_ps")
        for cc in range(CC_F):
            for nh in range(2):
                nc.tensor.matmul(
                    h1_ps[:, nh * 512:(nh + 1) * 512],
                    lhsT=xt_sb[:, cc, :],
                    rhs=wl_sb[:, cc, nh * 512:(nh + 1) * 512],
                    start=(cc == 0), stop=False)
        bias_mm(h1_ps, IB_BL)

        def ln_stats(src, lnw, rows=slice(0, 128)):
            """bn stats over the free dim: lnw holds [stats, mu/var, sd, rstd]."""
            st = lnw[:, 0:12].rearrange("p (g s) -> p g s", g=2)
            mv = lnw[:, 12:14]
            sd = lnw[:, 14:15]
            rstd = lnw[:, 15:16]
            src3 = src.rearrange("p (g d) -> p g d", g=2)
            for sg in range(2):
                nc.vector.bn_stats(out=st[rows, sg, :], in_=src3[rows, sg, :])
            nc.vector.bn_aggr(out=mv[rows, :], in_=st[rows, :, :])
            nc.scalar.activation(out=sd[rows, :], in_=mv[rows, 1:2],
                                 func=AF.Sqrt, bias=eps_t[rows, :], scale=1.0)
            nc.vector.reciprocal(out=rstd[rows, :], in_=sd[rows, :])
            return mv, rstd

        # ---- out1 = LN1(h1)*G + Bv  (LN applied from PSUM) ----
        lnw1 = scratch.tile([BC, 16], F32, tag="lnw", name="lnw1")
        mv, rstd = ln_stats(h1_ps, lnw1)
        nc.vector.tensor_scalar(out=out1, in0=h1_ps, scalar1=mv[:, 0:1],
                                scalar2=rstd, op0=ALU.subtract, op1=ALU.mult)
        nc.vector.tensor_mul(out1, out1, G_t)
        nc.vector.tensor_add(out1, out1, B_t)

        # ---- out1t = out1.T (per 128-chunk), for Wq matmul ----
        for cc in range(CC_F):
            tp = ps.tile([128, 128], F32, tag="s%d" % (cc % 2), bufs=1,
                         name="tp")
            nc.tensor.transpose(tp, out1[:, cc * 128:(cc + 1) * 128], ident)
            nc.vector.tensor_copy(out1t[:, cc, :], tp)

        # ---- q = (out1 @ Wq.T + bq)/sqrt(d), then transpose per head ----
        q_ps = ps.tile([BC, F_DIM], F32, tag="big", name="q_ps")
        for cc in range(CC_F):
            for nh in range(2):
                nc.tensor.matmul(
                    q_ps[:, nh * 512:(nh + 1) * 512],
                    lhsT=out1t[:, cc, :],
                    rhs=wq_sb[:, cc, nh * 512:(nh + 1) * 512],
                    start=(cc == 0), stop=False)
        bias_mm(q_ps, IB_BQS)
        q16 = s16p.tile([BC, F_DIM], BF16, tag="a16", name="q16")
        nc.vector.tensor_copy(q16, q_ps)
        for h in range(H):
            tp = ps.tile([128, 128], BF16, tag="s%d" % (h % 2), bufs=1,
                         name="tp")
            nc.tensor.transpose(tp, q16[:, h * 128:(h + 1) * 128], identb)
            nc.vector.tensor_copy(qt_sb[:, h, :], tp)

        # group 0 loads ride the scalar FIFO right behind the wq chunks
        tiles = {0: emit_dma(0, eng_c=nc.scalar, eng_t=nc.scalar),
                 1: emit_dma(1, eng_c=nc.scalar, eng_t=nc.scalar)}

        # ---- qkT[c, (h,b)] = Wk_h^T q_h : per-head projection of q ----
        for cc in range(CC_C):
            qk_p = ps.tile([128, F_DIM], F32, tag="big", name="qk_p")
            for h in range(H):
                nc.tensor.matmul(
                    qk_p[:, h * 128:(h + 1) * 128],
                    lhsT=wkh_sb[:, h, cc * 128:(cc + 1) * 128],
                    rhs=qt_sb[:, h, :],
                    start=True, stop=True)
            nc.vector.tensor_copy(
                qkT_sb[:, cc, :, :],
                qk_p.rearrange("p (h b) -> p h b", h=H))

        # group 1, then the tail weights, close out the scalar FIFO
        tiles[2] = emit_dma(2, eng_c=nc.scalar, eng_t=nc.scalar)
        tiles[3] = emit_dma(3, eng_c=nc.scalar, eng_t=nc.scalar)
        wvt_sb = singles.tile([128, CC_C, F_DIM], BF16)
        nc.scalar.dma_start(out=wvt_sb, in_=wvt)
        wo_sb = singles.tile([128, CC_F, F_DIM], BF16)
        nc.scalar.dma_start(out=wo_sb, in_=wo)

        # ---- main attention loop over 16-sample groups (software
        # pipelined: group g+1's score matmuls are emitted before group g's
        # softmax/amix so the PE never waits on the exp chain) ----

        def emit_scores(g):
            sc_ps = [ps.tile([128, 4 * T], F32, tag="s%d" % s, bufs=1,
                             name="sc_s%d" % s) for s in range(4)]
            for r in range(4):
                tfc_t = tiles[2 * g + r // 2][0]
                for cc in range(CC_C):
                    for s in range(4):
                        b = GS * g + 4 * r + s
                        nc.tensor.matmul(
                            sc_ps[s][32 * s:32 * s + 8, r * T:(r + 1) * T],
                            lhsT=qkT_sb[:, cc, :, b],
                            rhs=tfc_t[:, cc, 4 * (r % 2) + s, :],
                            start=(cc == 0), stop=(cc == CC_C - 1),
                            tile_position=(0, 32 * s))
            return sc_ps

        def emit_softmax(g, sc_ps):
            if att_mbg is not None:
                mb_t = attp.tile([128, 4 * T], F32, tag="mb")
                nc.sync.dma_start(out=mb_t, in_=att_mbg[g])
                for s in range(4):
                    pp = slice(32 * s, 32 * s + 8)
                    nc.vector.tensor_add(sc_ps[s][pp, :], sc_ps[s][pp, :],
                                         mb_t[pp, :])
            w_t = attp.tile([128, 4 * T], BF16, tag="w")
            den_t = attp.tile([128, 4], F32, tag="den")
            for s in range(4):
                pp = slice(32 * s, 32 * s + 8)
                nc.scalar.activation(out=w_t[pp, :], in_=sc_ps[s][pp, :],
                                     func=AF.Exp, bias=negm0_t[pp, :])
            nc.vector.tensor_reduce(
                out=den_t,
                in_=w_t.rearrange("p (r t) -> p r t", r=4),
                axis=mybir.AxisListType.X, op=ALU.add)
            rden_t = attp.tile([128, 4], F32, tag="rden")
            nc.vector.reciprocal(out=rden_t, in_=den_t)
            for r in range(4):
                nc.vector.tensor_scalar(
                    out=w_t[:, r * T:(r + 1) * T],
                    in0=w_t[:, r * T:(r + 1) * T],
                    scalar1=rden_t[:, r:r + 1], scalar2=None, op0=ALU.mult)
            at_sb = attp.tile([128, 4 * T], BF16, tag="at")
            # transposes borrow a "big" PSUM slot, NOT the score banks:
            # sharing the s* tags would chain amix(g) behind exp(g+1)
            # through the pool ring and serialize the loop.
            tp4 = ps.tile([128, 4, T], BF16, tag="big", name="tp4")
            for r in range(4):
                nc.tensor.transpose(tp4[:, r, :], w_t[:, r * T:(r + 1) * T],
                                    identb)
            nc.scalar.activation(
                out=at_sb.rearrange("p (r t) -> p r t", r=4), in_=tp4,
                func=AF.Copy)
            return at_sb

        def emit_amix(g, at_sb):
            # attention weights are the (8-col, cheap-LDW) stationary; tft
            # streams through as the moving operand.  4 samples run
            # concurrently in the PE column bands; each pass covers 4
            # samples -> [32q+h, c] rows, then 6 PE transposes collate to
            # the [c, b, h] layout the tail consumes.
            for p in range(4):
                pass_ps = ps.tile([128, TXT_DIM], F32, tag="big",
                                  name="pass_ps")
                for q in range(4):
                    bl = 4 * p + q
                    tft_t = tiles[2 * g + bl // HGS][1]
                    a0 = 128 * p + 32 * q
                    for c0, c1 in ((0, 512), (512, TXT_DIM)):
                        nc.tensor.matmul(
                            pass_ps[32 * q:32 * q + 8, c0:c1],
                            lhsT=at_sb[:, a0:a0 + 8],
                            rhs=tft_t[:, bl % HGS, c0:c1],
                            start=True, stop=True,
                            tile_position=(0, 32 * q))
                pass16 = s16p.tile([128, TXT_DIM], BF16, tag="a16",
                                   name="pass16")
                nc.vector.tensor_copy(pass16[:, 0:384], pass_ps[:, 0:384])
                nc.vector.tensor_copy(pass16[:, 384:768], pass_ps[:, 384:768])
                tp6 = ps.tile([128, CC_C, 128], BF16, tag="big", name="tp6")
                for cc in range(CC_C):
                    nc.tensor.transpose(
                        tp6[:, cc, :], pass16[:, cc * 128:(cc + 1) * 128],
                        identb)
                nc.scalar.activation(
                    out=amixT_sb[:, :, GS * g + 4 * p:GS * g + 4 * p + 4, :],
                    in_=tp6.rearrange("p cc (q x) -> p cc q x", q=4)
                    [:, :, :, 0:H],
                    func=AF.Copy)

        ao_sbs = {}

        def emit_tail_ctx(hf):
            """ctx + Wo matmuls for samples [64*hf, 64*(hf+1)); the ao
            result parks in SBUF so LN2/gelu can run after the loop (keeps
            SQRT/GELU ACT-table swaps off the loop's exp/copy tables)."""
            hb = slice(HB * hf, HB * (hf + 1))
            # ctxT[dv, (h, b)] = sum_c Wv_h[dv, c] * amix[b, h, c]
            ctxT_ps = ps.tile([128, H * HB], F32, tag="big", name="ctxT_ps")
            for h in range(H):
                for cc in range(CC_C):
                    nc.tensor.matmul(
                        ctxT_ps[:, h * HB:(h + 1) * HB],
                        lhsT=wvt_sb[:, cc, h * 128:(h + 1) * 128],
                        rhs=amixT_sb[:, cc, hb, h],
                        start=(cc == 0), stop=(cc == CC_C - 1))
            ctxT_sb = s16p.tile([128, H, HB], BF16, tag="a16", name="ctxT_sb")
            nc.vector.tensor_copy(
                ctxT_sb, ctxT_ps.rearrange("p (h b) -> p h b", h=H))
            # attn_out[b, f] = sum_h sum_dv ctxT[dv, h, b] Wo[f, h*128+dv]
            ao_ps = ps.tile([BC, F_DIM], F32, tag="big", name="ao_ps")
            for h in range(H):
                for nh in range(2):
                    nc.tensor.matmul(
                        ao_ps[hb, nh * 512:(nh + 1) * 512],
                        lhsT=ctxT_sb[:, h, :],
                        rhs=wo_sb[:, h, nh * 512:(nh + 1) * 512],
                        start=(h == 0), stop=False,
                        tile_position=(0, hb.start))
            bias_mm(ao_ps, IB_BO, part=hb)
            ao_sb = scratch.tile([BC, F_DIM], F32, tag="act", name="ao_sb")
            nc.vector.tensor_copy(ao_sb[hb, :], ao_ps[hb, :])
            ao_sbs[hf] = ao_sb

        def emit_tail_fin(hf):
            """LN2 + residual + gelu + output DMA for one half."""
            hb = slice(HB * hf, HB * (hf + 1))
            ao_sb = ao_sbs[hf]
            lnw2 = scratch.tile([BC, 16], F32, tag="lnw", name="lnw2")
            mv, rstd = ln_stats(ao_sb, lnw2, rows=hb)
            nc.vector.tensor_scalar(out=ao_sb[hb, :], in0=ao_sb[hb, :],
                                    scalar1=mv[hb, 0:1], scalar2=rstd[hb, :],
                                    op0=ALU.subtract, op1=ALU.mult)
            nc.vector.tensor_mul(ao_sb[hb, :], ao_sb[hb, :],
                                 gbt_sb[hb, 0, :])
            nc.vector.tensor_add(ao_sb[hb, :], ao_sb[hb, :],
                                 gbt_sb[hb, 1, :])
            nc.vector.tensor_add(ao_sb[hb, :], ao_sb[hb, :], out1[hb, :])
            out_sb = scratch.tile([BC, F_DIM], F32, tag="act", name="out_sb")
            nc.scalar.activation(out=out_sb[hb, :], in_=ao_sb[hb, :],
                                 func=AF.Gelu)
            nc.scalar.dma_start(out=out[hb, :], in_=out_sb[hb, :])

        sc_ps = {0: emit_scores(0)}
        for g in range(NG):
            for k in (2 * (g + 2), 2 * (g + 2) + 1):
                if k < 2 * NG:
                    tiles[k] = emit_dma(k)
            if g + 1 < NG:
                sc_ps[g + 1] = emit_scores(g + 1)
            at_sb = emit_softmax(g, sc_ps.pop(g))
            emit_amix(g, at_sb)
            if g == 3:
                emit_tail_ctx(0)
        emit_tail_ctx(1)
        emit_tail_fin(0)
        emit_tail_fin(1)


def _chunk_weight(w, n_cc, scale=None, dtype=np.float32, chunk_major=False):
    """W [F_out, C_in] -> device layout.

    chunk_major=False: [128, n_cc, F_out]  (p, cc, f) with c = cc*128+p
    chunk_major=True:  [n_cc, 128, F_out]
    """
    wt = w.T.astype(np.float32)
    if scale is not None:
        wt = wt * scale
    c_in, f_out = wt.shape
    assert c_in == n_cc * 128
    a = wt.reshape(n_cc, 128, f_out)
    if not chunk_major:
        a = a.transpose(1, 0, 2)
    return np.ascontiguousarray(a.astype(dtype))


def prep_inputs(x, z, text_feat, attention, Wg, bg, Wb, bb, Wl, bl, ln1_g,
                ln1_b, Wq, bq, Wk, bk, Wv, bv, Wo, bo, ln2_g, ln2_b,
                with_mask=False):
    """Build per-core input maps (list of 8 dicts of device-layout arrays)."""
    f32 = np.float32
    bf16 = ml_dtypes.bfloat16
    x = np.asarray(x, f32)
    z = np.asarray(z, f32)
    text_feat = np.asarray(text_feat, f32)
    attention = np.ascontiguousarray(np.asarray(attention, np.int32))

    # activations, per core
    xt = np.ascontiguousarray(
        x.reshape(NCORES, BC, CC_F, 128).transpose(0, 3, 2, 1).astype(bf16))
    zt = np.ascontiguousarray(
        z.reshape(NCORES, BC, CC_Z, 128).transpose(0, 3, 2, 1).astype(bf16))
    HGS = GS // 2
    tf16 = text_feat.astype(bf16).reshape(NCORES, NG, 2, HGS, T, CC_C, 128)
    # tfc[core][g][hf][p, cc, bl, t] = tf[16g+8hf+bl, t, cc*128+p]
    tfc = np.ascontiguousarray(tf16.transpose(0, 1, 2, 6, 5, 3, 4))
    # tft[core][g][hf][t, bl, c] = tf[16g+8hf+bl, t, c]
    tft = np.ascontiguousarray(
        text_feat.reshape(NCORES, NG, 2, HGS, T, TXT_DIM)
        .transpose(0, 1, 2, 4, 3, 5).astype(ml_dtypes.float8_e4m3))

    sD = 1.0 / np.sqrt(D)
    # wkh[d, h, c] = Wk[h*128+d, c]
    wkh = np.ascontiguousarray(
        np.asarray(Wk, f32).reshape(H, 128, TXT_DIM).transpose(1, 0, 2)
        .astype(bf16))
    # wvt[p, cc, hd] = Wv[hd, cc*128+p]
    wvt = np.ascontiguousarray(
        np.asarray(Wv, f32).T.reshape(CC_C, 128, F_DIM).transpose(1, 0, 2)
        .astype(bf16))
    shared = {
        "wg": _chunk_weight(Wg, CC_Z, dtype=bf16),
        "wb": _chunk_weight(Wb, CC_Z, dtype=bf16),
        "wl": _chunk_weight(Wl, CC_F, dtype=bf16),
        "wq": _chunk_weight(Wq, CC_F, scale=sD, dtype=bf16),
        "wo": _chunk_weight(Wo, CC_F, dtype=bf16),
        "wkh": wkh,
        "wvt": wvt,
        "bias": np.ascontiguousarray(np.stack([
            1.0 + np.asarray(bg, f32),
            np.asarray(bb, f32),
            np.asarray(bl, f32),
            np.asarray(bq, f32) * sD,
            np.asarray(bo, f32) + np.asarray(Wo, f32) @ np.asarray(bv, f32),
        ]).astype(bf16)),
        "gbt": np.ascontiguousarray(np.stack([
            np.asarray(ln1_g, f32),
            np.asarray(ln1_b, f32),
            np.asarray(ln2_g, f32),
            np.asarray(ln2_b, f32),
        ]).astype(bf16)),
    }
    in_maps = []
    for c in range(NCORES):
        m = dict(shared)
        m["xt"] = xt[c]
        m["zt"] = zt[c]
        m["tfc"] = tfc[c]
        m["tft"] = tft[c]
        if with_mask:
            # mbg[g][32s+h, r*T+t] = -1e30 where attention[16g+4r+s, t]==0
            att_c = attention.reshape(NCORES, BC, T)[c]
            mb = np.where(att_c != 0, 0.0, -1e30).astype(f32)  # [BC, T]
            mb = mb.reshape(NG, 4, 4, T)  # (g, r, s, t)
            mbg = np.zeros((NG, 128, 4 * T), f32)
            for s in range(4):
                for h in range(H):
                    mbg[:, 32 * s + h, :] = mb[:, :, s, :].reshape(NG, 4 * T)
            m["mbg"] = np.ascontiguousarray(mbg)
        in_maps.append(m)
    return in_maps


_CACHE = {}


def get_compiled(with_mask=False):
    key = ("nc", with_mask)
    if key not in _CACHE:
        nc = bacc.Bacc("TRN2", target_bir_lowering=False, debug=False,
                       enable_asserts=False)
        build(nc, with_mask=with_mask)
        nc.compile()
        _CACHE[key] = nc
    return _CACHE[key]


def run(in_maps, trace=False, with_mask=False, **kw):
    nc = get_compiled(with_mask=with_mask)
    return run_bass_kernel_spmd(nc, in_maps, list(range(NCORES)), trace=trace,
                                **kw)


def kernel(**inputs):
    with_mask = bool(np.any(np.asarray(inputs["attention"]) == 0))
    in_maps = prep_inputs(**inputs, with_mask=with_mask)
    res = run(in_maps, with_mask=with_mask)
    out = np.concatenate([res.results[c]["out"] for c in range(NCORES)],
                         axis=0)
    return np.ascontiguousarray(out.astype(np.float32))


if __name__ == "__main__":
    print("building + compiling...")
    get_compiled()
    print("done")


# revision 50
# speedup vs baseline: 1.1061x; 1.1061x over previous
"""Trainium2 Bass/Tile kernel for nn_CrossAttentionFiLM — factored attention.

Math (see reference):
    gamma = z @ Wg.T + bg ; beta = z @ Wb.T + bb
    out1  = LN1(x @ Wl.T + bl) * (1+gamma) + beta
    q     = (out1 @ Wq.T + bq) / sqrt(d)            [B, H, d]
    scores= qk . tf with qk = Wk_h^T q  (k never formed)
    attn  = softmax(scores)
    amix  = sum_t attn * tf ; ctx = Wv_h @ amix ; (+bv via folded bias)
    out   = gelu(out1 + LN2(ctx @ Wo.T + bo'))      bo' = bo + Wo@bv

Implementation notes (v2):
  - text_feat streams twice: tfc [c,t] bf16 (scores) on the sync HWDGE
    ring, tft [t,c] fp8 (amix) on the gpsimd SWDGE ring; weights and
    outputs ride the scalar HWDGE ring. Decoupled FIFOs keep the bulk
    stream saturated while the front chain's weights load.
  - biases are accumulated in PSUM by appending a rank-1 ones-matmul to
    each accumulation chain (no [128,F] broadcast DMAs).
  - LN gain/bias vectors are DMA-broadcast once and kept resident.
  - tail computes ctxT = Wv_h^T-contracted amix directly in [f, b]
    orientation (no PE transposes), runs Wo with ctxT stationary, and is
    split into two 64-sample halves so the first overlaps the group loop.
  - attention-weight transposes run in bf16 (quarter-bank PSUM tiles).

Sharding: pure data parallel over batch, B=1024 -> 128 rows per core x 8.
"""

import os
import sys

for _p in ("/opt/trn_rl_repo",):
    if os.path.isdir(_p) and _p not in sys.path:
        sys.path.append(_p)

import numpy as np
import ml_dtypes

os.environ.setdefault("JAX_COMPILATION_CACHE_DIR", "/tmp/jax_comp_cache")

import concourse.bass as bass
import concourse.tile as tile
from concourse import bacc, mybir
from concourse.bass_utils import run_bass_kernel_spmd
from concourse.masks import make_identity

BF16 = mybir.dt.bfloat16
FP8 = mybir.dt.float8e4
F32 = mybir.dt.float32
I32 = mybir.dt.int32
AF = mybir.ActivationFunctionType
ALU = mybir.AluOpType

B, T, F_DIM, Z_DIM, TXT_DIM, H = 1024, 128, 1024, 256, 768, 8
D = F_DIM // H  # 128
NCORES = 8
BC = B // NCORES  # 128 batch rows per core
EPS = 1e-5
CC_Z = Z_DIM // 128  # 2
CC_F = F_DIM // 128  # 8
CC_C = TXT_DIM // 128  # 6
NG = 8  # sample groups per core
GS = BC // NG  # 16 samples per group
HB = BC // 2  # tail half size (64 samples)

M0 = 16.0  # fixed softmax exp shift; |scores| ~ 12 max for this data

# bias table rows (all [F_DIM] f32 vectors; rank-1 matmul accumulated)
(IB_BG1, IB_BB, IB_BL, IB_BQS, IB_BO) = range(5)
# gbt rows (bf16, broadcast resident): ln1_g, ln1_b, ln2_g, ln2_b
(IG_G1, IG_B1, IG_G2, IG_B2) = range(4)


def build(nc, with_mask=False):
    """Declare per-core DRAM I/O and emit the Tile program."""
    xt = nc.dram_tensor("xt", [128, CC_F, BC], BF16, kind="ExternalInput").ap()
    zt = nc.dram_tensor("zt", [128, CC_Z, BC], BF16, kind="ExternalInput").ap()
    tfc = nc.dram_tensor("tfc", [NG, 2, 128, CC_C, GS // 2, T], BF16,
                         kind="ExternalInput").ap()
    tft = nc.dram_tensor("tft", [NG, 2, 128, GS // 2, TXT_DIM], FP8,
                         kind="ExternalInput").ap()
    wg = nc.dram_tensor("wg", [128, CC_Z, F_DIM], BF16, kind="ExternalInput").ap()
    wb = nc.dram_tensor("wb", [128, CC_Z, F_DIM], BF16, kind="ExternalInput").ap()
    wl = nc.dram_tensor("wl", [128, CC_F, F_DIM], BF16, kind="ExternalInput").ap()
    wq = nc.dram_tensor("wq", [128, CC_F, F_DIM], BF16, kind="ExternalInput").ap()
    wo = nc.dram_tensor("wo", [128, CC_F, F_DIM], BF16, kind="ExternalInput").ap()
    wkh = nc.dram_tensor("wkh", [128, H, TXT_DIM], BF16,
                         kind="ExternalInput").ap()
    wvt = nc.dram_tensor("wvt", [128, CC_C, F_DIM], BF16,
                         kind="ExternalInput").ap()
    bias = nc.dram_tensor("bias", [5, F_DIM], BF16, kind="ExternalInput").ap()
    gbt = nc.dram_tensor("gbt", [4, F_DIM], BF16, kind="ExternalInput").ap()
    mbg = None
    if with_mask:
        mbg = nc.dram_tensor("mbg", [NG, 128, 4 * T], F32,
                             kind="ExternalInput").ap()
    out = nc.dram_tensor("out", [BC, F_DIM], F32, kind="ExternalOutput").ap()

    with tile.TileContext(nc) as tc:
        _emit(nc, tc, xt, zt, tfc, tft, att_mbg=mbg, wg=wg, wb=wb, wl=wl,
              wq=wq, wo=wo, wkh=wkh, wvt=wvt, bias=bias, gbt=gbt, out=out)
    return nc


def _emit(nc, tc, xt, zt, tfc, tft, att_mbg, wg, wb, wl, wq, wo, wkh, wvt,
          bias, gbt, out):
    from contextlib import ExitStack

    ctxmgr = ExitStack()
    with ctxmgr:
        singles = ctxmgr.enter_context(tc.tile_pool(name="singles", bufs=1))
        scratch = ctxmgr.enter_context(tc.tile_pool(name="scratch", bufs=3))
        s16p = ctxmgr.enter_context(tc.tile_pool(name="s16p", bufs=4))
        wstream = ctxmgr.enter_context(tc.tile_pool(name="wstream", bufs=2))
        tfcp = ctxmgr.enter_context(tc.tile_pool(name="tfcp", bufs=2))
        tftp = ctxmgr.enter_context(tc.tile_pool(name="tftp", bufs=2))
        attp = ctxmgr.enter_context(tc.tile_pool(name="attp", bufs=2))
        # PSUM: 8 banks: "big" [128,1024]x2bufs = 4 banks, four single-bank
        # score tags "s0".."s3" (concurrent accumulation chains must live in
        # different banks).  bf16 PE transposes share the "s*" slots.
        ps = ctxmgr.enter_context(tc.tile_pool(name="ps", bufs=2, space="PSUM"))

        # ---- DMA ordering: everything front-critical rides the scalar
        # HWDGE queue as ONE priority FIFO (SDMA round-robin is packet-fair,
        # so a concurrent bulk stream would starve the small weight loads).
        # Groups 0/1 also load from the scalar queue; groups 2+ ride the
        # sync (tfc) and gpsimd (tft) rings, naturally gated by the tile
        # pools' bufs=2 recycling so they cannot start before group-0 is
        # consumed. ----
        # text_feat streams in HALF-group (8-sample) slices: finer pool
        # gating keeps the bulk queues saturated and PE idle gaps short.
        HGS = GS // 2

        def emit_dma(k, eng_c=None, eng_t=None):
            g, hf = k // 2, k % 2
            tfc_t = tfcp.tile([128, CC_C, HGS, T], BF16, tag="tfc", bufs=3,
                              name="tfc_t")
            (eng_c or nc.sync).dma_start(out=tfc_t, in_=tfc[g][hf])
            tft_t = tftp.tile([128, HGS, TXT_DIM], FP8, tag="tft", bufs=3,
                              name="tft_t")
            (eng_t or nc.gpsimd).dma_start(out=tft_t, in_=tft[g][hf])
            return tfc_t, tft_t

        # bias rows live at 32-aligned partitions (PE operand alignment):
        # row i -> partition 32*(i%4), free slot i//4
        bias_sb = singles.tile([128, 2, F_DIM], BF16)
        nc.scalar.dma_start(
            out=bias_sb[0:97:32, 0, :],
            in_=bias[0:4, :])
        nc.scalar.dma_start(out=bias_sb[0:1, 1, :], in_=bias[4:5, :])
        zt_sb = singles.tile([128, CC_Z, BC], BF16)
        nc.scalar.dma_start(out=zt_sb, in_=zt)
        wg_sb = singles.tile([128, CC_Z, F_DIM], BF16)
        nc.scalar.dma_start(out=wg_sb, in_=wg)
        wb_sb = singles.tile([128, CC_Z, F_DIM], BF16)
        nc.scalar.dma_start(out=wb_sb, in_=wb)
        xt_sb = singles.tile([128, CC_F, BC], BF16)
        nc.scalar.dma_start(out=xt_sb, in_=xt)
        wkh_sb = singles.tile([128, H, TXT_DIM], BF16)
        nc.scalar.dma_start(out=wkh_sb, in_=wkh)
        # ln2 g/b stay resident; ln1 g/b ride transient s16p tiles (SBUF)
        gbt_sb = singles.tile([128, 2, F_DIM], BF16)
        g2 = gbt[2:4, :]
        nc.scalar.dma_start(
            out=gbt_sb,
            in_=bass.AP(tensor=g2.tensor, offset=g2.offset,
                        ap=[[0, 128]] + list(g2.ap)))
        g1b1_t = s16p.tile([128, 2, F_DIM], BF16, tag="g1b1", bufs=1,
                           name="g1b1_t")
        g1 = gbt[0:2, :]
        nc.scalar.dma_start(
            out=g1b1_t,
            in_=bass.AP(tensor=g1.tensor, offset=g1.offset,
                        ap=[[0, 128]] + list(g1.ap)))
        wl_sb = singles.tile([128, CC_F, F_DIM], BF16)
        nc.scalar.dma_start(out=wl_sb, in_=wl)
        wq_sb = singles.tile([128, CC_F, F_DIM], BF16)
        nc.scalar.dma_start(out=wq_sb, in_=wq)
        ident = singles.tile([128, 128], F32)
        make_identity(nc, ident)
        identb = singles.tile([128, 128], BF16)
        nc.gpsimd.tensor_copy(identb, ident)
        eps_t = singles.tile([128, 1], F32)
        nc.vector.memset(eps_t, EPS)
        negm0_t = singles.tile([128, 1], F32)
        nc.vector.memset(negm0_t, -M0)
        ones_t = singles.tile([128, 128], BF16)
        nc.vector.memset(ones_t, 1.0)

        def bias_mm(ps_t, row, part=slice(0, 128)):
            """Accumulate bias row into an open PSUM chain (rank-1 matmul)."""
            n = part.stop - part.start
            p0 = 32 * (row % 4)
            for nh in range(2):
                nc.tensor.matmul(
                    ps_t[part, nh * 512:(nh + 1) * 512],
                    lhsT=ones_t[p0:p0 + 1, 0:n],
                    rhs=bias_sb[p0:p0 + 1, row // 4,
                                nh * 512:(nh + 1) * 512],
                    start=False, stop=True,
                    tile_position=(p0, part.start))

        # persistent activations
        out1 = singles.tile([BC, F_DIM], F32)
        out1t = singles.tile([128, CC_F, BC], BF16)
        qt_sb = singles.tile([128, H, BC], BF16)
        qkT_sb = singles.tile([128, CC_C, H, BC], BF16)
        # amixT is kept b-major (cc, b, h) so the per-group PSUM->SBUF copy
        # is contiguous; the tail's matmul reads it with an 8-stride AP.
        amixT_sb = singles.tile([128, CC_C, BC, H], BF16)

        # ---- FiLM params: gamma1 = z@Wg.T + (1+bg), beta = z@Wb.T + bb ----
        film16 = {}
        for w_sb, brow, nm in ((wg_sb, IB_BG1, "gamma1"), (wb_sb, IB_BB, "beta")):
            ps_t = ps.tile([BC, F_DIM], F32, tag="big", name="film_ps")
            for cc in range(CC_Z):
                for nh in range(2):
                    nc.tensor.matmul(
                        ps_t[:, nh * 512:(nh + 1) * 512],
                        lhsT=zt_sb[:, cc, :],
                        rhs=w_sb[:, cc, nh * 512:(nh + 1) * 512],
                        start=(cc == 0), stop=False)
            bias_mm(ps_t, brow)
            f16 = s16p.tile([BC, F_DIM], BF16, tag="a16", name=nm)
            nc.vector.tensor_copy(f16, ps_t)
            film16[nm] = f16

        # FiLM-folded LN1 gain/bias: G = g1*(1+gamma), Bv = b1*(1+gamma)+beta
        G_t = s16p.tile([BC, F_DIM], BF16, tag="a16", name="G_t")
        nc.vector.tensor_mul(G_t, film16["gamma1"], g1b1_t[:, 0, :])
        B_t = s16p.tile([BC, F_DIM], BF16, tag="a16", name="B_t")
        nc.vector.tensor_mul(B_t, film16["gamma1"], g1b1_t[:, 1, :])
        nc.vector.tensor_add(B_t, B_t, film16["beta"])

        # ---- h1 = x@Wl.T + bl (PSUM-resident) ----
        h1_ps = ps.tile([BC, F_DIM], F32, tag="big", name="h1_ps")
        for cc in range(CC_F):
            for nh in range(2):
                nc.tensor.matmul(
                    h1_ps[:, nh * 512:(nh + 1) * 512],
                    lhsT=xt_sb[:, cc, :],
                    rhs=wl_sb[:, cc, nh * 512:(nh + 1) * 512],
                    start=(cc == 0), stop=False)
        bias_mm(h1_ps, IB_BL)

        def ln_stats(src, lnw, rows=slice(0, 128)):
            """bn stats over the free dim: lnw holds [stats, mu/var, sd, rstd]."""
            st = lnw[:, 0:12].rearrange("p (g s) -> p g s", g=2)
            mv = lnw[:, 12:14]
            sd = lnw[:, 14:15]
            rstd = lnw[:, 15:16]
            src3 = src.rearrange("p (g d) -> p g d", g=2)
            for sg in range(2):
                nc.vector.bn_stats(out=st[rows, sg, :], in_=src3[rows, sg, :])
            nc.vector.bn_aggr(out=mv[rows, :], in_=st[rows, :, :])
            nc.scalar.activation(out=sd[rows, :], in_=mv[rows, 1:2],
                                 func=AF.Sqrt, bias=eps_t[rows, :], scale=1.0)
            nc.vector.reciprocal(out=rstd[rows, :], in_=sd[rows, :])
            return mv, rstd

        # ---- out1 = LN1(h1)*G + Bv  (LN applied from PSUM) ----
        lnw1 = scratch.tile([BC, 16], F32, tag="lnw", name="lnw1")
        mv, rstd = ln_stats(h1_ps, lnw1)
        nc.vector.tensor_scalar(out=out1, in0=h1_ps, scalar1=mv[:, 0:1],
                                scalar2=rstd, op0=ALU.subtract, op1=ALU.mult)
        nc.vector.tensor_mul(out1, out1, G_t)
        nc.vector.tensor_add(out1, out1, B_t)

        # ---- out1t = out1.T (per 128-chunk), for Wq matmul ----
        for cc in range(CC_F):
            tp = ps.tile([128, 128], F32, tag="s%d" % (cc % 2), bufs=1,
                         name="tp")
            nc.tensor.transpose(tp, out1[:, cc * 128:(cc + 1) * 128], ident)
            nc.vector.tensor_copy(out1t[:, cc, :], tp)

        # ---- q = (out1 @ Wq.T + bq)/sqrt(d), then transpose per head ----
        q_ps = ps.tile([BC, F_DIM], F32, tag="big", name="q_ps")
        for cc in range(CC_F):
            for nh in range(2):
                nc.tensor.matmul(
                    q_ps[:, nh * 512:(nh + 1) * 512],
                    lhsT=out1t[:, cc, :],
                    rhs=wq_sb[:, cc, nh * 512:(nh + 1) * 512],
                    start=(cc == 0), stop=False)
        bias_mm(q_ps, IB_BQS)
        q16 = s16p.tile([BC, F_DIM], BF16, tag="a16", name="q16")
        nc.vector.tensor_copy(q16, q_ps)
        for h in range(H):
            tp = ps.tile([128, 128], BF16, tag="s%d" % (h % 2), bufs=1,
                         name="tp")
            nc.tensor.transpose(tp, q16[:, h * 128:(h + 1) * 128], identb)
            nc.vector.tensor_copy(qt_sb[:, h, :], tp)

        # group 0 loads ride the scalar FIFO right behind the wq chunks
        tiles = {0: emit_dma(0, eng_c=nc.scalar, eng_t=nc.scalar),
                 1: emit_dma(1, eng_c=nc.scalar, eng_t=nc.scalar)}

        # ---- qkT[c, (h,b)] = Wk_h^T q_h : per-head projection of q ----
        for cc in range(CC_C):
            qk_p = ps.tile([128, F_DIM], F32, tag="big", name="qk_p")
            for h in range(H):
                nc.tensor.matmul(
                    qk_p[:, h * 128:(h + 1) * 128],
                    lhsT=wkh_sb[:, h, cc * 128:(cc + 1) * 128],
                    rhs=qt_sb[:, h, :],
                    start=True, stop=True)
            nc.vector.tensor_copy(
                qkT_sb[:, cc, :, :],
                qk_p.rearrange("p (h b) -> p h b", h=H))

        # group 1, then the tail weights, close out the scalar FIFO
        tiles[2] = emit_dma(2, eng_c=nc.scalar, eng_t=nc.scalar)
        tiles[3] = emit_dma(3, eng_c=nc.scalar, eng_t=nc.scalar)
        wvt_sb = singles.tile([128, CC_C, F_DIM], BF16)
        nc.scalar.dma_start(out=wvt_sb, in_=wvt)
        wo_sb = singles.tile([128, CC_F, F_DIM], BF16)
        nc.scalar.dma_start(out=wo_sb, in_=wo)

        # ---- main attention loop over 16-sample groups (software
        # pipelined: group g+1's score matmuls are emitted before group g's
        # softmax/amix so the PE never waits on the exp chain) ----

        def emit_scores(g):
            sc_ps = [ps.tile([128, 4 * T], F32, tag="s%d" % s, bufs=1,
                             name="sc_s%d" % s) for s in range(4)]
            for r in range(4):
                tfc_t = tiles[2 * g + r // 2][0]
                for cc in range(CC_C):
                    for s in range(4):
                        b = GS * g + 4 * r + s
                        nc.tensor.matmul(
                            sc_ps[s][32 * s:32 * s + 8, r * T:(r + 1) * T],
                            lhsT=qkT_sb[:, cc, :, b],
                            rhs=tfc_t[:, cc, 4 * (r % 2) + s, :],
                            start=(cc == 0), stop=(cc == CC_C - 1),
                            tile_position=(0, 32 * s))
            return sc_ps

        def emit_softmax(g, sc_ps):
            if att_mbg is not None:
                mb_t = attp.tile([128, 4 * T], F32, tag="mb")
                nc.sync.dma_start(out=mb_t, in_=att_mbg[g])
                for s in range(4):
                    pp = slice(32 * s, 32 * s + 8)
                    nc.vector.tensor_add(sc_ps[s][pp, :], sc_ps[s][pp, :],
                                         mb_t[pp, :])
            w_t = attp.tile([128, 4 * T], BF16, tag="w")
            den_t = attp.tile([128, 4], F32, tag="den")
            for s in range(4):
                pp = slice(32 * s, 32 * s + 8)
                nc.scalar.activation(out=w_t[pp, :], in_=sc_ps[s][pp, :],
                                     func=AF.Exp, bias=negm0_t[pp, :])
            nc.vector.tensor_reduce(
                out=den_t,
                in_=w_t.rearrange("p (r t) -> p r t", r=4),
                axis=mybir.AxisListType.X, op=ALU.add)
            rden_t = attp.tile([128, 4], F32, tag="rden")
            nc.vector.reciprocal(out=rden_t, in_=den_t)
            for r in range(4):
                nc.vector.tensor_scalar(
                    out=w_t[:, r * T:(r + 1) * T],
                    in0=w_t[:, r * T:(r + 1) * T],
                    scalar1=rden_t[:, r:r + 1], scalar2=None, op0=ALU.mult)
            at_sb = attp.tile([128, 4 * T], BF16, tag="at")
            # transposes borrow a "big" PSUM slot, NOT the score banks:
            # sharing the s* tags would chain amix(g) behind exp(g+1)
            # through the pool ring and serialize the loop.
            tp4 = ps.tile([128, 4, T], BF16, tag="big", name="tp4")
            for r in range(4):
                nc.tensor.transpose(tp4[:, r, :], w_t[:, r * T:(r + 1) * T],
                                    identb)
            nc.scalar.activation(
                out=at_sb.rearrange("p (r t) -> p r t", r=4), in_=tp4,
                func=AF.Copy)
            return at_sb

        def emit_amix(g, at_sb):
            ax_p = ps.tile([128, F_DIM], F32, tag="big", name="ax_p")
            for bl in range(GS):
                c0 = (bl // 4) * T + (bl % 4) * 32
                tft_t = tiles[2 * g + bl // HGS][1]
                for cc in range(CC_C):
                    nc.tensor.matmul(
                        ax_p[:, cc * 128 + bl * 8:cc * 128 + bl * 8 + 8],
                        lhsT=tft_t[:, bl % HGS, cc * 128:(cc + 1) * 128],
                        rhs=at_sb[:, c0:c0 + 8],
                        start=True, stop=True)
            nc.scalar.activation(
                out=amixT_sb[:, :, GS * g:GS * (g + 1), :],
                in_=ax_p[:, 0:CC_C * 128].rearrange(
                    "p (cc b h) -> p cc b h", cc=CC_C, b=GS),
                func=AF.Copy)

        ao_sbs = {}

        def emit_tail_ctx(hf):
            """ctx + Wo matmuls for samples [64*hf, 64*(hf+1)); the ao
            result parks in SBUF so LN2/gelu can run after the loop (keeps
            SQRT/GELU ACT-table swaps off the loop's exp/copy tables)."""
            hb = slice(HB * hf, HB * (hf + 1))
            # ctxT[dv, (h, b)] = sum_c Wv_h[dv, c] * amix[b, h, c]
            ctxT_ps = ps.tile([128, H * HB], F32, tag="big", name="ctxT_ps")
            for h in range(H):
                for cc in range(CC_C):
                    nc.tensor.matmul(
                        ctxT_ps[:, h * HB:(h + 1) * HB],
                        lhsT=wvt_sb[:, cc, h * 128:(h + 1) * 128],
                        rhs=amixT_sb[:, cc, hb, h],
                        start=(cc == 0), stop=(cc == CC_C - 1))
            ctxT_sb = s16p.tile([128, H, HB], BF16, tag="a16", name="ctxT_sb")
            nc.vector.tensor_copy(
                ctxT_sb, ctxT_ps.rearrange("p (h b) -> p h b", h=H))
            # attn_out[b, f] = sum_h sum_dv ctxT[dv, h, b] Wo[f, h*128+dv]
            ao_ps = ps.tile([BC, F_DIM], F32, tag="big", name="ao_ps")
            for h in range(H):
                for nh in range(2):
                    nc.tensor.matmul(
                        ao_ps[hb, nh * 512:(nh + 1) * 512],
                        lhsT=ctxT_sb[:, h, :],
                        rhs=wo_sb[:, h, nh * 512:(nh + 1) * 512],
                        start=(h == 0), stop=False,
                        tile_position=(0, hb.start))
            bias_mm(ao_ps, IB_BO, part=hb)
            ao_sb = scratch.tile([BC, F_DIM], F32, tag="act", name="ao_sb")
            nc.vector.tensor_copy(ao_sb[hb, :], ao_ps[hb, :])
            ao_sbs[hf] = ao_sb

        def emit_tail_fin(hf):
            """LN2 + residual + gelu + output DMA for one half."""
            hb = slice(HB * hf, HB * (hf + 1))
            ao_sb = ao_sbs[hf]
            lnw2 = scratch.tile([BC, 16], F32, tag="lnw", name="lnw2")
            mv, rstd = ln_stats(ao_sb, lnw2, rows=hb)
            nc.vector.tensor_scalar(out=ao_sb[hb, :], in0=ao_sb[hb, :],
                                    scalar1=mv[hb, 0:1], scalar2=rstd[hb, :],
                                    op0=ALU.subtract, op1=ALU.mult)
            nc.vector.tensor_mul(ao_sb[hb, :], ao_sb[hb, :],
                                 gbt_sb[hb, 0, :])
            nc.vector.tensor_add(ao_sb[hb, :], ao_sb[hb, :],
                                 gbt_sb[hb, 1, :])
            nc.vector.tensor_add(ao_sb[hb, :], ao_sb[hb, :], out1[hb, :])
            out_sb = scratch.tile([BC, F_DIM], F32, tag="act", name="out_sb")
            nc.scalar.activation(out=out_sb[hb, :], in_=ao_sb[hb, :],
                                 func=AF.Gelu)
            nc.scalar.dma_start(out=out[hb, :], in_=out_sb[hb, :])

        sc_ps = {0: emit_scores(0)}
        for g in range(NG):
            for k in (2 * (g + 2), 2 * (g + 2) + 1):
                if k < 2 * NG:
                    tiles[k] = emit_dma(k)
            if g + 1 < NG:
                sc_ps[g + 1] = emit_scores(g + 1)
            at_sb = emit_softmax(g, sc_ps.pop(g))
            emit_amix(g, at_sb)
            if g == 3:
                emit_tail_ctx(0)
        emit_tail_ctx(1)
        emit_tail_fin(0)
        emit_tail_fin(1)


def _chunk_weight(w, n_cc, scale=None, dtype=np.float32, chunk_major=False):
    """W [F_out, C_in] -> device layout.

    chunk_major=False: [128, n_cc, F_out]  (p, cc, f) with c = cc*128+p
    chunk_major=True:  [n_cc, 128, F_out]
    """
    wt = w.T.astype(np.float32)
    if scale is not None:
        wt = wt * scale
    c_in, f_out = wt.shape
    assert c_in == n_cc * 128
    a = wt.reshape(n_cc, 128, f_out)
    if not chunk_major:
        a = a.transpose(1, 0, 2)
    return np.ascontiguousarray(a.astype(dtype))


def prep_inputs(x, z, text_feat, attention, Wg, bg, Wb, bb, Wl, bl, ln1_g,
                ln1_b, Wq, bq, Wk, bk, Wv, bv, Wo, bo, ln2_g, ln2_b,
                with_mask=False):
    """Build per-core input maps (list of 8 dicts of device-layout arrays)."""
    f32 = np.float32
    bf16 = ml_dtypes.bfloat16
    x = np.asarray(x, f32)
    z = np.asarray(z, f32)
    text_feat = np.asarray(text_feat, f32)
    attention = np.ascontiguousarray(np.asarray(attention, np.int32))

    # activations, per core
    xt = np.ascontiguousarray(
        x.reshape(NCORES, BC, CC_F, 128).transpose(0, 3, 2, 1).astype(bf16))
    zt = np.ascontiguousarray(
        z.reshape(NCORES, BC, CC_Z, 128).transpose(0, 3, 2, 1).astype(bf16))
    HGS = GS // 2
    tf16 = text_feat.astype(bf16).reshape(NCORES, NG, 2, HGS, T, CC_C, 128)
    # tfc[core][g][hf][p, cc, bl, t] = tf[16g+8hf+bl, t, cc*128+p]
    tfc = np.ascontiguousarray(tf16.transpose(0, 1, 2, 6, 5, 3, 4))
    # tft[core][g][hf][t, bl, c] = tf[16g+8hf+bl, t, c]
    tft = np.ascontiguousarray(
        text_feat.reshape(NCORES, NG, 2, HGS, T, TXT_DIM)
        .transpose(0, 1, 2, 4, 3, 5).astype(ml_dtypes.float8_e4m3))

    sD = 1.0 / np.sqrt(D)
    # wkh[d, h, c] = Wk[h*128+d, c]
    wkh = np.ascontiguousarray(
        np.asarray(Wk, f32).reshape(H, 128, TXT_DIM).transpose(1, 0, 2)
        .astype(bf16))
    # wvt[p, cc, hd] = Wv[hd, cc*128+p]
    wvt = np.ascontiguousarray(
        np.asarray(Wv, f32).T.reshape(CC_C, 128, F_DIM).transpose(1, 0, 2)
        .astype(bf16))
    shared = {
        "wg": _chunk_weight(Wg, CC_Z, dtype=bf16),
        "wb": _chunk_weight(Wb, CC_Z, dtype=bf16),
        "wl": _chunk_weight(Wl, CC_F, dtype=bf16),
        "wq": _chunk_weight(Wq, CC_F, scale=sD, dtype=bf16),
        "wo": _chunk_weight(Wo, CC_F, dtype=bf16),
        "wkh": wkh,
        "wvt": wvt,
        "bias": np.ascontiguousarray(np.stack([
            1.0 + np.asarray(bg, f32),
            np.asarray(bb, f32),
            np.asarray(bl, f32),
            np.asarray(bq, f32) * sD,
            np.asarray(bo, f32) + np.asarray(Wo, f32) @ np.asarray(bv, f32),
        ]).astype(bf16)),
        "gbt": np.ascontiguousarray(np.stack([
            np.asarray(ln1_g, f32),
            np.asarray(ln1_b, f32),
            np.asarray(ln2_g, f32),
            np.asarray(ln2_b, f32),
        ]).astype(bf16)),
    }
    in_maps = []
    for c in range(NCORES):
        m = dict(shared)
        m["xt"] = xt[c]
        m["zt"] = zt[c]
        m["tfc"] = tfc[c]
        m["tft"] = tft[c]
        if with_mask:
            # mbg[g][32s+h, r*T+t] = -1e30 where attention[16g+4r+s, t]==0
            att_c = attention.reshape(NCORES, BC, T)[c]
            mb = np.where(att_c != 0, 0.0, -1e30).astype(f32)  # [BC, T]
            mb = mb.reshape(NG, 4, 4, T)  # (g, r, s, t)
            mbg = np.zeros((NG, 128, 4 * T), f32)
            for s in range(4):
                for h in range(H):
                    mbg[:, 32 * s + h, :] = mb[:, :, s, :].reshape(NG, 4 * T)
            m["mbg"] = np.ascontiguousarray(mbg)
        in_maps.append(m)
    return in_maps


_CACHE = {}


def get_compiled(with_mask=False):
    key = ("nc", with_mask)
    if key not in _CACHE:
        nc = bacc.Bacc("TRN2", target_bir_lowering=False, debug=False,
                       enable_asserts=False)
        build(nc, with_mask=with_mask)
        nc.compile()
        _CACHE[key] = nc
    return _CACHE[key]


def run(in_maps, trace=False, with_mask=False, **kw):
    nc = get_compiled(with_mask=with_mask)
    return run_bass_kernel_spmd(nc, in_maps, list(range(NCORES)), trace=trace,
                                **kw)


def kernel(**inputs):
    with_mask = bool(np.any(np.asarray(inputs["attention"]) == 0))
    in_maps = prep_inputs(**inputs, with_mask=with_mask)
    res = run(in_maps, with_mask=with_mask)
    out = np.concatenate([res.results[c]["out"] for c in range(NCORES)],
                         axis=0)
    return np.ascontiguousarray(out.astype(np.float32))


if __name__ == "__main__":
    print("building + compiling...")
    get_compiled()
    print("done")
